# revision 7
# baseline (speedup 1.0000x reference)
"""Additive attention (B=8, Q=K=512, H=Dv=64) on 8 TRN2 NeuronCores.

Math per batch b (reference):
    qf = queries @ Wq; kf = keys @ Wk
    scores[q,k] = sum_h wv[h] * tanh(qf[q,h] + kf[k,h])   (k >= valid_len masked)
    out = softmax_k(scores) @ values

Key idea: replace the pointwise tanh (134M ScalarEngine evaluations, ~93us)
with a low-rank bilinear expansion
    tanh(a+b) ~= sum_r phi_r(a) * psi_r(b),   r < R=10
obtained from the SVD of the kernel tanh(a+b) discretized on a grid with
sqrt-Gaussian row/column weighting (qf,kf entries are ~N(0,1)). Then
    scores[q,k] = sum_{r,h} Phi[q, r*64+h] * Psi[k, r*64+h]
is a plain matmul with contraction F = R*64 = 640 done on the PE.

Sharding: data-parallel, one batch per core. Host computes qf/kf (0.4% of
FLOPs), evaluates the R basis functions per element (table interp), packs
features into 128-row contraction chunks (2 ranks x 64 h); ranks 0-1 ship
bf16, ranks 2-9 fp8(e4m3) with per-rank scale balancing (rank errors scale
with the decaying singular values). The key-side softmax mask is FOLDED
into the features: the (rank 1, argmin|wv|) slot is repurposed as
Phi=1 / Psi = 0 or -60000, so masked columns get score ~ -6e4 and exp -> 0
with no per-partition bias needed (the stolen slot's term is ~|wv|_min,
negligible). Device per core: 16 chunk-matmuls accumulate scores^T
[4 k-tiles x 128, 512q] into one 4-bank PSUM tile (fp8 chunk pairs use
DoubleRow, 2 contraction chunks per instruction), ONE merged exp over all
4 banks -> p bf16, 4 values-matmuls (ones column -> denominator row),
copy + DMA out. Host divides and transposes. Early dummy matmuls keep the
PE p-state ramp warm while inputs stream.
"""
import numpy as np
import ml_dtypes

B = 8
Q = 512
K = 512
H = 64
DV = 64

R = 10                 # SVD rank of tanh(a+b)
NBF = 1                # bf16 chunks (2 ranks each): ranks 0..1
NF8 = 4                # fp8 chunks: ranks 2..9 (DoubleRow pairs)
NCH = NBF + NF8
NTILE = K // 128       # 4 k-tiles
MASKBIG = -60000.0
F8MAX = 224.0          # ml_dtypes.float8_e4m3 max finite is 240
WARMUP_MM = 7          # PE p-state ramp fillers while input DMA streams

GRID_N, GRID_A, GRID_SIG, GRID_FLOOR = 1201, 6.5, 1.15, 0.02

_BASIS = None


def _basis():
    """SVD basis of tanh(a+b) on a weighted grid: x, phi[n,R], psi[n,R]."""
    global _BASIS
    if _BASIS is None:
        x = np.linspace(-GRID_A, GRID_A, GRID_N)
        Kg = np.tanh(x[:, None] + x[None, :])
        w = np.sqrt(np.exp(-x ** 2 / (2 * GRID_SIG ** 2))) + GRID_FLOOR
        U, S, Vt = np.linalg.svd((w[:, None] * Kg) * w[None, :])
        phi = (U[:, :R] * np.sqrt(S[:R])) / w[:, None]
        psi = (Vt[:R].T * np.sqrt(S[:R])) / w[:, None]
        _BASIS = (x, phi, psi)
    return _BASIS


# ---------------------------------------------------------------------------
# BIR post-pass: the walrus build in this environment accepts only one
# sync-wait command per instruction; hoist extras onto same-engine NoOps.
def _split_waits(nc, k=1):
    import concourse.mybir as mybir
    n_new = 0
    for f in nc.m.functions:
        for bb in f.blocks:
            newlist = []
            for ins in bb.instructions:
                si = ins.sync_info
                if si is not None and si.on_wait and len(si.on_wait) > k:
                    waits = list(si.on_wait)
                    extra, keep = waits[:-k], waits[-k:]
                    for ci, w in enumerate(extra):
                        nop = mybir.InstNoOp(
                            name=f"{ins.name}_wsplit{ci}",
                            engine=ins.engine,
                            ins=[], outs=[],
                            sync_info=mybir.SyncInfo(on_wait=[w], on_update=[]),
                        )
                        newlist.append(nop)
                        n_new += 1
                    ins.sync_info = mybir.SyncInfo(
                        on_wait=list(keep), on_update=list(si.on_update))
                newlist.append(ins)
            bb.instructions[:] = newlist
    return n_new


def _build(nc, reps: int = 1):
    import concourse.bass as bass  # noqa: F401
    import concourse.mybir as mybir
    from concourse import tile

    F32 = mybir.dt.float32
    BF16 = mybir.dt.bfloat16
    F8 = mybir.dt.float8e4
    DR = mybir.MatmulPerfMode.DoubleRow

    fb16 = nc.declare_dram_parameter("fb16", [NBF * 2 * 128, Q], BF16,
                                     isOutput=False)
    f8d = nc.declare_dram_parameter("f8d", [NF8 * 2 * 128, Q], F8,
                                    isOutput=False)
    vl1 = nc.declare_dram_parameter("vl1", [128, NTILE * (DV + 1)], BF16,
                                    isOutput=False)
    o65 = nc.declare_dram_parameter("o65", [DV + 1, Q], F32, isOutput=True)

    # constant-source broadcast APs (pre-initialized at Bass init; lets the
    # PE warm up with no memset dependency)
    cb = nc.const_aps.aps[(BF16, 1.0)]
    warm_rhs = bass.AP(cb.tensor, cb.offset, [[1, 1], [0, Q]])
    warm_lhsT = bass.AP(cb.tensor, cb.offset, [[1, 1], [0, 16]])

    with tile.TileContext(nc) as tc:  # noqa: F841
        with (
            tc.tile_pool(name="cpool", bufs=1) as cpool,
            tc.tile_pool(name="ppool", bufs=2) as ppool,
            tc.tile_pool(name="ps_s", bufs=1, space="PSUM") as ps_s,
            tc.tile_pool(name="ps_o", bufs=1, space="PSUM") as ps_o,
            tc.tile_pool(name="ps_w", bufs=1, space="PSUM") as ps_w,
        ):
            # PE p-state warmup + ACT exp-table prefetch during input DMA
            psw = ps_w.tile([16, Q], F32, tag="warm", name="psw")
            for i in range(WARMUP_MM):
                nc.tensor.matmul(psw[:], warm_lhsT, warm_rhs,
                                 start=True, stop=True)
            dummy = cpool.tile([1, 16], F32)
            nc.scalar.activation(
                dummy[:], bass.AP(cb.tensor, cb.offset, [[1, 1], [0, 16]]),
                mybir.ActivationFunctionType.Exp)

            for rep in range(reps):
                fb16_sb = cpool.tile([128, NBF * 2, Q], BF16,
                                     tag="fb16", name=f"fb16_{rep}")
                f8_sb = cpool.tile([128, NF8 // 2, 4, Q], F8,
                                   tag="f8", name=f"f8_{rep}")
                vl_sb = cpool.tile([128, NTILE, DV + 1], BF16,
                                   tag="vl", name=f"vl_{rep}")
                nc.sync.dma_start(
                    fb16_sb[:], fb16.rearrange("(c p) n -> p c n", p=128))
                for g in range(NF8 // 2):
                    nc.sync.dma_start(
                        f8_sb[:, g],
                        f8d.rearrange("(g c p) n -> g p c n",
                                      g=NF8 // 2, p=128)[g])
                nc.sync.dma_start(
                    vl_sb[:], vl1.rearrange("p (t d) -> p t d", t=NTILE))

                scA = ps_s.tile([128, 2, Q], F32, tag="scA",
                                name=f"scA_{rep}")
                scB = ps_s.tile([128, 2, Q], F32, tag="scB",
                                name=f"scB_{rep}")
                sct = lambda t: (scA if t < 2 else scB)[:, t % 2, :]
                po = ps_o.tile([DV + 1, Q], F32, tag="po", name=f"po_{rep}")

                # bf16 chunks, chunk-major so compute chases the DMA stream
                for c in range(NBF):
                    for t in range(NTILE):
                        nc.tensor.matmul(
                            sct(t),
                            fb16_sb[:, 2 * c, t * 128:(t + 1) * 128],
                            fb16_sb[:, 2 * c + 1, :],
                            start=(c == 0), stop=False)
                # fp8 DoubleRow groups (2 contraction chunks per matmul);
                # exp per k-tile pair emitted right after its last stop so
                # its sem wait covers only the matmuls it needs
                pA = ppool.tile([128, 2, Q], BF16, tag="pA", name=f"pA_{rep}")
                pB = ppool.tile([128, 2, Q], BF16, tag="pB", name=f"pB_{rep}")
                for g in range(NF8 // 2):
                    last = g == NF8 // 2 - 1
                    for t in range(NTILE):
                        nc.tensor.matmul(
                            sct(t),
                            f8_sb[:, g, 0:2, t * 128:(t + 1) * 128],
                            f8_sb[:, g, 2:4, :],
                            start=False, stop=last, perf_mode=DR)
                        if last and t == 1:
                            nc.scalar.activation(
                                pA[:], scA[:],
                                mybir.ActivationFunctionType.Exp)
                    if last:
                        nc.scalar.activation(
                            pB[:], scB[:],
                            mybir.ActivationFunctionType.Exp)
                for t in range(NTILE):
                    p = pA if t < 2 else pB
                    nc.tensor.matmul(
                        po[:], vl_sb[:, t, :], p[:, t % 2, :],
                        start=(t == 0), stop=(t == NTILE - 1))
                # split the copy across Pool/DVE and ship halves on separate
                # queues so the fixed DMA paths overlap
                HQ = Q // 2
                o65a = cpool.tile([DV + 1, HQ], F32,
                                  tag="o65a", name=f"o65a_{rep}")
                o65b = cpool.tile([DV + 1, HQ], F32,
                                  tag="o65b", name=f"o65b_{rep}")
                nc.scalar.activation(o65a[:], po[:, 0:HQ],
                                     mybir.ActivationFunctionType.Copy)
                nc.vector.tensor_copy(o65b[:], po[:, HQ:Q])
                nc.gpsimd.dma_start(o65[:, 0:HQ], o65a[:])
                nc.sync.dma_start(o65[:, HQ:Q], o65b[:])
    return nc


def host_inputs(queries, keys, values, valid_lens, Wq, Wk, wv):
    x, phi, psi = _basis()
    queries = np.asarray(queries, np.float32)
    keys = np.asarray(keys, np.float32)
    values = np.asarray(values, np.float32)
    wv = np.asarray(wv, np.float32)
    qf = (queries @ np.asarray(Wq, np.float32)).astype(np.float32)  # [B,Q,H]
    kf = (keys @ np.asarray(Wk, np.float32)).astype(np.float32)     # [B,K,H]
    hmin = int(np.argmin(np.abs(wv)))

    maps = []
    for b in range(B):
        Phi = np.stack([np.interp(qf[b], x, phi[:, r]) for r in range(R)],
                       1).astype(np.float32)              # [Q, R, H]
        Psi = np.stack([np.interp(kf[b], x, psi[:, r]) for r in range(R)],
                       1).astype(np.float32) * wv         # [K, R, H]
        mxq = np.abs(Phi).max(axis=(0, 2))
        mxk = np.abs(Psi).max(axis=(0, 2))
        alpha = np.sqrt(np.maximum(mxk, 1e-30) / np.maximum(mxq, 1e-30))
        Phi *= alpha[None, :, None]
        Psi /= alpha[None, :, None]
        # fold the key mask into the (rank 2*NBF-1, argmin|wv|) slot
        L = int(valid_lens[b])
        rm = 2 * NBF - 1
        Phi[:, rm, hmin] = 1.0
        Psi[:, rm, hmin] = np.where(np.arange(K) < L, 0.0, MASKBIG)
        # chunk c = ranks (2c, 2c+1): contraction row = 64*(r-2c) + h
        PhiT = Phi.reshape(Q, R * H).T      # [F, Q]
        PsiT = Psi.reshape(K, R * H).T      # [F, K]
        blocks16 = []
        for c in range(NBF):
            blocks16 += [PsiT[c * 128:(c + 1) * 128],
                         PhiT[c * 128:(c + 1) * 128]]
        fb = np.concatenate(blocks16, 0).astype(ml_dtypes.bfloat16)
        blocks8 = []
        for g in range(NF8 // 2):
            c0 = NBF + 2 * g
            blocks8 += [PsiT[c0 * 128:(c0 + 1) * 128],
                        PsiT[(c0 + 1) * 128:(c0 + 2) * 128],
                        PhiT[c0 * 128:(c0 + 1) * 128],
                        PhiT[(c0 + 1) * 128:(c0 + 2) * 128]]
        f8 = np.clip(np.concatenate(blocks8, 0), -F8MAX, F8MAX)
        f8 = f8.astype(ml_dtypes.float8_e4m3)

        vla = np.zeros((128, NTILE * (DV + 1)), np.float32)
        for t in range(NTILE):
            vla[:, t * (DV + 1):t * (DV + 1) + DV] = \
                values[b][t * 128:(t + 1) * 128]
            vla[:, t * (DV + 1) + DV] = 1.0
        maps.append({
            "fb16": fb,
            "f8d": f8,
            "vl1": vla.astype(ml_dtypes.bfloat16),
        })
    return maps


def host_merge(results):
    out = np.empty((B, Q, DV), np.float32)
    for b in range(B):
        o = np.asarray(results[b]["o65"], np.float32)   # [65, Q]
        out[b] = (o[0:DV] / o[DV][None, :]).T
    return np.ascontiguousarray(out)


_RUNNER = None


def _get_runner():
    """Build + compile once per process; returns a callable(in_maps)->results."""
    global _RUNNER
    if _RUNNER is not None:
        return _RUNNER
    import jax
    from jax.sharding import Mesh, PartitionSpec
    from jax.experimental.shard_map import shard_map
    import concourse.bass as bass
    import concourse.mybir as mybir
    from concourse import bass2jax
    from concourse.bass2jax import _bass_exec_p, install_neuronx_cc_hook

    nc = bass.Bass()
    _build(nc)
    _split_waits(nc)

    install_neuronx_cc_hook()
    partition_name = nc.partition_id_tensor.name if nc.partition_id_tensor else None
    in_names, out_names, out_avals, zero_shapes = [], [], [], []
    for alloc in nc.m.functions[0].allocations:
        if not isinstance(alloc, mybir.MemoryLocationSet):
            continue
        name = alloc.memorylocations[0].name
        if alloc.kind == "ExternalInput":
            if name != partition_name:
                in_names.append(name)
        elif alloc.kind == "ExternalOutput":
            out_names.append(name)
            shape = tuple(alloc.tensor_shape)
            dtype = mybir.dt.np(alloc.dtype)
            out_avals.append(jax.core.ShapedArray(shape, dtype))
            zero_shapes.append((shape, dtype))
    n_params = len(in_names)
    n_outs = len(out_avals)
    in_names_all = in_names + out_names
    if partition_name is not None:
        in_names_all.append(partition_name)
    donate = tuple(range(n_params, n_params + n_outs))

    def _body(*args):
        operands = list(args)
        if partition_name is not None:
            operands.append(bass2jax.partition_id_tensor())
        outs = _bass_exec_p.bind(
            *operands,
            out_avals=tuple(out_avals),
            in_names=tuple(in_names_all),
            out_names=tuple(out_names),
            lowering_input_output_aliases=(),
            sim_require_finite=True,
            sim_require_nnan=True,
            nc=nc,
        )
        return tuple(outs)

    devices = jax.devices()[:8]
    mesh = Mesh(np.asarray(devices), ("core",))
    in_specs = (PartitionSpec("core"),) * (n_params + n_outs)
    out_specs = (PartitionSpec("core"),) * len(out_names)
    sharded = jax.jit(
        shard_map(_body, mesh=mesh, in_specs=in_specs, out_specs=out_specs,
                  check_rep=False),
        donate_argnums=donate, keep_unused=True,
    )

    def run(in_maps):
        per_core = [[np.asarray(m[name]) for name in in_names] for m in in_maps]
        concat_in = [
            np.concatenate([per_core[c][i] for c in range(8)], axis=0)
            for i in range(n_params)
        ]
        zeros = [np.zeros((8 * s[0],) + s[1:], d) for s, d in zero_shapes]
        out_arrs = sharded(*concat_in, *zeros)
        out_np = [np.asarray(a) for a in out_arrs]
        return [
            {name: out_np[i].reshape(8, *out_avals[i].shape)[c]
             for i, name in enumerate(out_names)}
            for c in range(8)
        ]

    _RUNNER = run
    return run


def kernel(queries, keys, values, valid_lens, Wq, Wk, wv):
    run = _get_runner()
    in_maps = host_inputs(queries, keys, values, valid_lens, Wq, Wk, wv)
    try:
        results = run(in_maps)
    except Exception:
        # transient NRT/axon failures have been observed; retry once
        results = run(in_maps)
    return host_merge(results)
